# revision 11
# baseline (speedup 1.0000x reference)
"""Angles2BasisDihedral Trainium2 kernel (8 NeuronCores, data-parallel).

Math: per sample b with angles alpha/beta (L=512), per-position rotation
  A_j = Rz(alpha_j) @ Rx(beta_j)  (3x3), cumulative M_p = A_1 @ ... @ A_p,
  output[b, r, 3p+c] = M_p[r][c] for p=0..L (M_0 = I), zeroed for p > len_b.

Device strategy (per core, 2048 samples = 16 blocks of 128 partitions):
  - host pre-wraps angles into [-pi, pi] (ACT Sin table is only accurate there)
    and pre-sorts samples by length, striped across the 8 cores; per-block
    chunk counts are baked into the instruction stream at build time.
  - masking is folded into the A entries: A_j = 0 for j > len_b, which makes
    every masked prefix product exactly zero.
  - chunked scan: C<=16 chunks of S=32 along the chain; phase 1 computes local
    prefixes for all chunks in parallel (batch in partitions, chunks in the
    free dim), phase 2 ripples the 3x3 carries across chunks, phase 3 applies
    carries to all local prefixes and writes rows in the final output layout.
  - engine-linear dataflow DMA -> ACT -> DVE -> DMA keeps every instruction at
    <=1 cross-engine semaphore wait (TRN2 ISA limit).
"""
import math
import numpy as np

import concourse.bacc as bacc
import concourse.mybir as mybir
from concourse.bass_utils import run_bass_kernel_spmd
from concourse.tile import TileContext

B, L = 16384, 512
NCORES = 8
BPC = B // NCORES            # samples per core (2048)
NBLK = BPC // 128            # 16 blocks of 128 partitions
S = 32                       # chunk length
CMAX = L // S                # 16 chunks
OUTW = 3 * (L + 1)           # 1539 columns per row
INW = 4 * L + 8              # wa | wac | wb | wbc | len+0.5 | pad
F32 = mybir.dt.float32
ADD = mybir.AluOpType.add
MULT = mybir.AluOpType.mult

LAST_EXEC_NS = None
_CACHE = {}


def _build(cbs):
    """Build the Bass program for per-block chunk counts `cbs` (len NBLK)."""
    nc = bacc.Bacc("TRN2", target_bir_lowering=False)
    # const needed for activation scale=-1.0
    t = nc.alloc_sbuf_tensor("const-f32-neg1", [128, 1], F32)
    nc.gpsimd.memset(t.ap(), -1.0)
    nc.const_aps.aps[(F32, -1.0)] = t.ap()
    nc.all_engine_barrier()

    inp = nc.declare_dram_parameter("inp", [BPC, INW], F32, isOutput=False)
    cst = nc.declare_dram_parameter("cst", [128, L], F32, isOutput=False)
    out = nc.declare_dram_parameter("out", [BPC, 3 * OUTW], F32, isOutput=True)

    SIN = mybir.ActivationFunctionType.Sin
    SIGN = mybir.ActivationFunctionType.Sign
    RELU = mybir.ActivationFunctionType.Relu
    IDT = mybir.ActivationFunctionType.Identity

    with TileContext(nc) as tc:
        with (
            tc.tile_pool(name="pcst", bufs=1) as pcst,
            tc.tile_pool(name="pin", bufs=2) as pin,
            tc.tile_pool(name="ptrig", bufs=2) as ptrig,
            tc.tile_pool(name="pwork", bufs=1) as pwork,
            tc.tile_pool(name="pout", bufs=2) as pout,
        ):
            iota = pcst.tile([128, L], F32)
            nc.gpsimd.dma_start(out=iota[:, :], in_=cst[:, :])
            # ACT warmup: absorb the const-DMA semaphore into ACT's clock
            warm = pcst.tile([128, 1], F32)
            nc.scalar.activation(warm[:, :], iota[:, 0:1], IDT)

            for b in range(NBLK):
                cb = cbs[b]
                ot = pout.tile([128, 3 * OUTW], F32, tag="ot")
                # identity frame at position 0 + zero tails (first writers of
                # the recycled out tile -> they carry the WAR wait on its DMA)
                NP = cb * S
                for r in range(3):
                    base = r * OUTW
                    nc.vector.memset(ot[:, base:base + 3], 0.0)
                    if 3 + 3 * NP < OUTW:
                        nc.vector.memset(ot[:, base + 3 + 3 * NP:base + OUTW], 0.0)
                for r in range(3):
                    nc.vector.memset(ot[:, r * OUTW + r:r * OUTW + r + 1], 1.0)

                if cb == 0:
                    nc.gpsimd.dma_start(out=out[b * 128:(b + 1) * 128, :], in_=ot[:, :])
                    continue

                it = pin.tile([128, INW], F32, tag="it")
                nc.gpsimd.dma_start(out=it[:, :], in_=inp[b * 128:(b + 1) * 128, :])
                lens = it[:, 4 * L:4 * L + 1]

                tg = ptrig.tile([128, 8 * S * CMAX], F32, tag="tg")
                sgn = tg[:, 0 * L:0 * L + NP]
                m01 = tg[:, 1 * L:1 * L + NP]
                sa = tg[:, 2 * L:2 * L + NP]
                ca = tg[:, 3 * L:3 * L + NP]
                nsa = tg[:, 4 * L:4 * L + NP]
                nca = tg[:, 5 * L:5 * L + NP]
                sb = tg[:, 6 * L:6 * L + NP]
                cb_ = tg[:, 7 * L:7 * L + NP]

                # absorber: first ACT write to the recycled trig tile carries
                # only the WAR dep (DVE readers of the old buffer)
                nc.scalar.activation(tg[:, 0:1], iota[:, 0:1], IDT)
                # ACT chain (first op joins the input DMA via the bias AP;
                # iota was already observed through the warmup op)
                nc.scalar.activation(sgn, iota[:, 0:NP], SIGN, bias=lens, scale=-1.0)
                nc.scalar.activation(m01, sgn, RELU)
                nc.scalar.activation(sa, it[:, 0:NP], SIN)
                nc.scalar.activation(ca, it[:, L:L + NP], SIN)
                nc.scalar.activation(nsa, it[:, 0:NP], SIN, scale=-1.0)
                nc.scalar.activation(nca, it[:, L:L + NP], SIN, scale=-1.0)
                nc.scalar.activation(sb, it[:, 2 * L:2 * L + NP], SIN)
                nc.scalar.activation(cb_, it[:, 3 * L:3 * L + NP], SIN)

                # A tile: [pos][c][m] (slot e = c*3+m), masked entries
                A = pwork.tile([128, 9 * S * CMAX], F32, tag="A")
                Av = A[:, 0:9 * NP].rearrange("p (s e) -> p s e", s=NP, e=9)
                nc.vector.tensor_tensor(out=Av[:, :, 0], in0=ca, in1=m01, op=MULT)
                nc.vector.tensor_tensor(out=Av[:, :, 1], in0=sa, in1=m01, op=MULT)
                nc.vector.tensor_tensor(out=Av[:, :, 5], in0=sb, in1=m01, op=MULT)
                nc.vector.tensor_tensor(out=Av[:, :, 8], in0=cb_, in1=m01, op=MULT)
                nc.vector.memset(Av[:, :, 2], 0.0)
                # products (one masked factor suffices)
                nc.vector.tensor_tensor(out=Av[:, :, 4], in0=ca, in1=Av[:, :, 8], op=MULT)
                nc.vector.tensor_tensor(out=Av[:, :, 6], in0=sa, in1=Av[:, :, 5], op=MULT)
                nc.vector.tensor_tensor(out=Av[:, :, 3], in0=nsa, in1=Av[:, :, 8], op=MULT)
                nc.vector.tensor_tensor(out=Av[:, :, 7], in0=nca, in1=Av[:, :, 5], op=MULT)

                # ---- phase 1: local prefix scans, all chunks in parallel ----
                # P layout: [ch][r][jj][c]  (strides 9S / 3S / 3 / 1)
                P = pwork.tile([128, 9 * S * CMAX], F32, tag="P")
                P5 = P[:, 0:9 * NP].rearrange("p (ch r jj c) -> p ch r jj c",
                                              ch=cb, r=3, jj=S, c=3)
                A5 = A[:, 0:9 * NP].rearrange("p (ch jj c m) -> p ch jj c m",
                                              ch=cb, jj=S, c=3, m=3)
                prods = pwork.tile([128, 27 * CMAX], F32, tag="prods")
                pr5 = prods[:, 0:27 * cb].rearrange("p (ch r c m) -> p ch r c m",
                                                    ch=cb, r=3, c=3, m=3)
                # step 0: P[ch][r][0][c] = A_entry[r][c] (A holds [c][m=r])
                nc.vector.tensor_copy(out=P5[:, :, :, 0, :],
                                      in_=A5[:, :, 0].transpose([0, 1, 3, 2]))
                for jj in range(1, S):
                    in0 = (P5[:, :, :, jj - 1, :].unsqueeze(3)
                           .broadcast_to([128, cb, 3, 3, 3]))
                    in1 = (A5[:, :, jj].unsqueeze(2)
                           .broadcast_to([128, cb, 3, 3, 3]))
                    nc.vector.tensor_tensor(out=pr5, in0=in0, in1=in1, op=MULT)
                    nc.vector.tensor_tensor(out=P5[:, :, :, jj, :],
                                            in0=pr5[:, :, :, :, 0],
                                            in1=pr5[:, :, :, :, 1], op=ADD)
                    nc.vector.tensor_tensor(out=P5[:, :, :, jj, :],
                                            in0=P5[:, :, :, jj, :],
                                            in1=pr5[:, :, :, :, 2], op=ADD)

                if cb > 1:
                    # ---- phase 2: ripple carries across chunks ----
                    carry = pwork.tile([128, 9 * CMAX], F32, tag="carry")
                    c4 = carry[:, 0:9 * cb].rearrange("p (ch r c) -> p ch r c",
                                                      ch=cb, r=3, c=3)
                    cp = pwork.tile([128, 27], F32, tag="cp")
                    cp4 = cp[:, :].rearrange("p (r c m) -> p r c m", r=3, c=3, m=3)
                    # carry[0] = P[0][.][S-1][.]
                    nc.vector.tensor_copy(out=c4[:, 0], in_=P5[:, 0, :, S - 1, :])
                    for ch in range(1, cb):
                        in0 = c4[:, ch - 1].unsqueeze(2).broadcast_to([128, 3, 3, 3])
                        in1 = (P5[:, ch, :, S - 1, :].transpose([0, 2, 1])
                               .unsqueeze(1).broadcast_to([128, 3, 3, 3]))
                        nc.vector.tensor_tensor(out=cp4, in0=in0, in1=in1, op=MULT)
                        nc.vector.tensor_tensor(out=c4[:, ch], in0=cp4[:, :, :, 0],
                                                in1=cp4[:, :, :, 1], op=ADD)
                        nc.vector.tensor_tensor(out=c4[:, ch], in0=c4[:, ch],
                                                in1=cp4[:, :, :, 2], op=ADD)

                    # ---- phase 3: apply carries, write final layout ----
                    p3 = pwork.tile([128, 9 * S * (CMAX - 1)], F32, tag="p3")
                    JC = 3 * S
                    p3m = p3[:, 0:9 * S * (cb - 1)].rearrange(
                        "p (ch jc m) -> p ch jc m", ch=cb - 1, jc=JC, m=3)
                    p35 = p3[:, 0:9 * S * (cb - 1)].rearrange(
                        "p (ch jj c m) -> p ch jj c m", ch=cb - 1, jj=S, c=3, m=3)
                    for r in range(3):
                        in0 = (c4[:, 0:cb - 1, r, :].unsqueeze(2)
                               .broadcast_to([128, cb - 1, JC, 3]))
                        in1 = P5[:, 1:cb].rearrange("p ch m jj c -> p ch (jj c) m")
                        nc.vector.tensor_tensor(out=p3m, in0=in0, in1=in1, op=MULT)
                        ov = (ot[:, r * OUTW + 3:r * OUTW + 3 + 3 * NP]
                              .rearrange("p (ch jj c) -> p ch jj c", ch=cb, jj=S, c=3))
                        nc.vector.tensor_tensor(out=ov[:, 1:cb], in0=p35[:, :, :, :, 0],
                                                in1=p35[:, :, :, :, 1], op=ADD)
                        nc.vector.tensor_tensor(out=ov[:, 1:cb], in0=ov[:, 1:cb],
                                                in1=p35[:, :, :, :, 2], op=ADD)

                # chunk 0 rows straight from P
                for r in range(3):
                    ov = (ot[:, r * OUTW + 3:r * OUTW + 3 + 3 * S]
                          .rearrange("p (jj c) -> p jj c", jj=S, c=3))
                    nc.vector.tensor_copy(out=ov, in_=P5[:, 0, r])

                nc.gpsimd.dma_start(out=out[b * 128:(b + 1) * 128, :], in_=ot[:, :])

    nc.finalize()
    return nc


def _wrap(x):
    return x - (2.0 * np.pi) * np.round(x / (2.0 * np.pi))


def prepare(input, angles_length):
    """Host-side prep: sort/stripe/wrap inputs, build (cached) Bass program."""
    input = np.asarray(input, dtype=np.float32)
    lens = np.asarray(angles_length).astype(np.int64)

    order = np.argsort(lens, kind="stable")
    in_maps = []
    core_lens = []
    for k in range(NCORES):
        idx = order[k::NCORES]
        core_lens.append(lens[idx])
        a = input[idx, 0, :]
        bta = input[idx, 1, :]
        arr = np.empty((BPC, INW), dtype=np.float32)
        arr[:, 0:L] = _wrap(a)
        arr[:, L:2 * L] = _wrap(a + np.pi / 2)
        arr[:, 2 * L:3 * L] = _wrap(bta)
        arr[:, 3 * L:4 * L] = _wrap(bta + np.pi / 2)
        arr[:, 4 * L] = core_lens[k] + 0.5
        arr[:, 4 * L + 1:] = 0.0
        in_maps.append({"inp": arr})

    iota = np.broadcast_to(np.arange(1, L + 1, dtype=np.float32), (128, L)).copy()
    for m in in_maps:
        m["cst"] = iota

    # per-block chunk counts: max over cores of the block's max length
    cbs = []
    for b_ in range(NBLK):
        mx = max(int(core_lens[k][(b_ + 1) * 128 - 1]) for k in range(NCORES))
        cbs.append(min(CMAX, math.ceil(mx / S)) if mx > 0 else 0)
    key = tuple(cbs)
    if key not in _CACHE:
        _CACHE[key] = _build(cbs)
    nc = _CACHE[key]
    return nc, in_maps, order


def kernel(input, angles_length):
    nc, in_maps, order = prepare(input, angles_length)
    res = run_bass_kernel_spmd(nc, in_maps, core_ids=list(range(NCORES)))
    full = np.empty((B, 3, OUTW), dtype=np.float32)
    for k in range(NCORES):
        idx = order[k::NCORES]
        full[idx] = res.results[k]["out"].reshape(BPC, 3, OUTW)
    return full
